# revision 1
# baseline (speedup 1.0000x reference)
"""BarrierNet Trainium2 kernel: MLP (6->128->128x2 branches->heads) + closed-form QP.

Data-parallel over 8 cores (16384 samples each). Host pre-shards and
pre-transposes: xt [6,NS] bf16, weights pre-transposed bf16 (heads padded to
M=32 col-groups), and broadcast-constant pattern tiles. Per core:
  - MLP in transposed layout (hidden on partitions, batch free), 8 chunks of
    2048; each layer = 4 bf16 N=512 matmuls into a 4-bank PSUM group drained
    by ONE fused bias+tanh ACT op (fp32 PSUM -> bf16 SBUF).
  - Heads: 4 chunks' [3,512]/[2,512] outputs packed into one PSUM bank via
    tile_position col-groups -> cheap FD=512 DVE copies.
  - Head outputs round-trip through DRAM scratch into sample-grid layout
    [128, j] where the QP math runs batched full-width on DVE in fp32.

All DMAs issue from the (otherwise idle) Pool/GPSIMD engine via SWDGE.
"""
import sys

sys.path.insert(0, "/opt/trn_rl_repo")

import numpy as np
import ml_dtypes

import concourse.bacc as bacc
import concourse.bass as bass
import concourse.mybir as mybir
import concourse.tile as tile
from concourse import bass_utils

FP = mybir.dt.float32
BF = mybir.dt.bfloat16
AF = mybir.ActivationFunctionType
OP = mybir.AluOpType
BF_NP = ml_dtypes.bfloat16

N_CORES = 8
B = 131072
NS = B // N_CORES          # samples per core
SC = 2048                  # super-chunk (one PSUM group span)
H = 128
NF = 6

_cache = {}


def build(ns=NS):
    nit = ns // SC
    NQ = 2
    jh = ns // NQ // 128   # samples per partition in one QP grid
    J3, J2, J6 = 3 * jh, 2 * jh, 6 * jh

    nc = bacc.Bacc("TRN2", target_bir_lowering=False, debug=False)

    x_d = nc.dram_tensor("x", [ns, NF], FP, kind="ExternalInput")
    xt_d = nc.dram_tensor("xt", [NF, ns], BF, kind="ExternalInput")
    w1T_d = nc.dram_tensor("w1T", [NF, H], BF, kind="ExternalInput")
    w21T_d = nc.dram_tensor("w21T", [H, H], BF, kind="ExternalInput")
    w22T_d = nc.dram_tensor("w22T", [H, H], BF, kind="ExternalInput")
    wm1T_d = nc.dram_tensor("wm1T", [H, H], BF, kind="ExternalInput")
    wm2T_d = nc.dram_tensor("wm2T", [H, H], BF, kind="ExternalInput")
    wh31_d = nc.dram_tensor("wh31", [H, 32], BF, kind="ExternalInput")
    wh32_d = nc.dram_tensor("wh32", [H, 32], BF, kind="ExternalInput")
    b1_d = nc.dram_tensor("b1", [H], FP, kind="ExternalInput")
    b21_d = nc.dram_tensor("b21", [H], FP, kind="ExternalInput")
    b22_d = nc.dram_tensor("b22", [H], FP, kind="ExternalInput")
    bm1_d = nc.dram_tensor("bm1", [H], FP, kind="ExternalInput")
    bm2_d = nc.dram_tensor("bm2", [H], FP, kind="ExternalInput")
    # qc columns: 0:J6 stdg | J6:2*J6 moffg | 2*J6:2*J6+J3 b31g | +J2 b32g
    QW = 2 * J6 + J3 + J2
    qc_d = nc.dram_tensor("qc", [H, QW], FP, kind="ExternalInput")
    u_d = nc.dram_tensor("u", [ns, 3], FP, kind="ExternalOutput")

    dma = nc.gpsimd.dma_start

    with tile.TileContext(nc) as tc:
        with (
            tc.tile_pool(name="const", bufs=1) as cpool,
            tc.tile_pool(name="act", bufs=2) as apool,
            tc.tile_pool(name="hd", bufs=2) as hpool,
            tc.tile_pool(name="psum", bufs=2, space="PSUM") as ppool,
            tc.tile_pool(name="qp", bufs=2) as qpool,
            tc.tile_pool(name="qtmp", bufs=2) as tpool,
            tc.tile_pool(name="dram", bufs=1, space="DRAM") as dpool,
        ):
            # ---------------- load weights / constants (all contiguous) -----
            w1T = cpool.tile([NF, H], BF, tag="w1T", name="w1T")
            nc.sync.dma_start(w1T[:], w1T_d[:])
            wh31 = cpool.tile([H, 32], BF, tag="wh31", name="wh31")
            dma(wh31[:], wh31_d[:])
            wh32 = cpool.tile([H, 32], BF, tag="wh32", name="wh32")
            dma(wh32[:], wh32_d[:])
            wts = {}
            for nm, wd in (("w21", w21T_d), ("w22", w22T_d),
                           ("wm1", wm1T_d), ("wm2", wm2T_d)):
                wt = cpool.tile([H, H], BF, tag=nm + "T")
                dma(wt[:], wd[:])
                wts[nm] = wt
            bias = {}
            for nm, bd in (("b1", b1_d), ("b21", b21_d), ("b22", b22_d),
                           ("bm1", bm1_d), ("bm2", bm2_d)):
                bt = cpool.tile([H, 1], FP, tag=nm)
                (nc.sync.dma_start if nm == "b1" else dma)(
                    bt[:], bd.rearrange("(p o) -> p o", o=1))
                bias[nm] = bt
            qc = cpool.tile([H, QW], FP, tag="qc", name="qc")
            dma(qc[:], qc_d[:])

            scr = [dpool.tile([5, ns // NQ], FP, tag=f"scr{h}", name=f"scr{h}")
                   for h in range(NQ)]

            # ---------------- QP (sample-grid layout, fp32, batched) --------
            QS = {}

            def T(hh, tag, w):
                t = tpool.tile([128, w], FP, tag=tag, name=tag)
                QS[hh][tag] = t[:]
                return t[:]

            def qp_pre(hh):
                """x-side preamble: needs only x_d — emitted early, runs on
                idle DVE during the fc1/fc21/fc22 phases."""
                QS[hh] = {}
                g0 = hh * (ns // NQ)
                xg = qpool.tile([128, J6], FP, tag="xg", name="xg")
                nc.sync.dma_start(xg[:],
                    x_d[g0 : g0 + ns // NQ, :].rearrange("(p j) f -> p (j f)", p=128))
                V = nc.vector
                xgv = xg.rearrange("p (j g e) -> p e g j", g=3, e=2)
                x0 = T(hh, "x0", J6)
                x0v = x0.rearrange("p (e g j) -> p e g j", e=2, g=3)
                qsv = qc[:, 0:J6].rearrange("p (e g j) -> p e g j", e=2, g=3)
                qmv = qc[:, J6 : 2 * J6].rearrange("p (e g j) -> p e g j", e=2, g=3)
                V.tensor_mul(x0v, xgv, qsv)
                V.tensor_add(x0v, x0v, qmv)
                dd, vv = x0[:, 0:J3], x0[:, J3:J6]
                d2 = T(hh, "d2", J3); V.tensor_mul(d2, dd, dd)
                d3 = T(hh, "d3", J3); V.tensor_mul(d3, d2, dd)
                d4 = T(hh, "d4", J3); V.tensor_mul(d4, d2, d2)
                v2 = T(hh, "v2", J3); V.tensor_mul(v2, vv, vv)
                dv = T(hh, "dv", J3); V.tensor_mul(dv, d3, vv)
                dw = T(hh, "dw", J3); V.tensor_mul(dw, d2, v2)
                d6 = T(hh, "d6", J3); V.tensor_mul(d6, d3, d3)

                def a3(t, k):
                    return t[:, k * jh : (k + 1) * jh]

                def sum3(t, tag, bias_const=None):
                    r = T(hh, tag, jh)
                    V.tensor_add(r, a3(t, 0), a3(t, 1))
                    if bias_const is None:
                        V.tensor_add(r, r, a3(t, 2))
                    else:
                        V.scalar_tensor_tensor(r, r, bias_const, a3(t, 2),
                                               OP.add, OP.add)
                    return r

                bar = sum3(d4, "bar", -2401.0)   # barrier
                bd = sum3(dv, "bd")              # barrier_dot / 4
                Ls = sum3(dw, "Ls")              # Lf2b / 12
                g6 = sum3(d6, "g6")              # GG / 16
                rg = T(hh, "rg", jh); V.reciprocal(rg, g6)

            def qp_post(hh):
                """head-side chain: needs the scr round-trip."""
                g0 = hh * (ns // NQ)
                V = nc.vector
                q_ = QS[hh]
                hg = qpool.tile([128, 5 * jh], FP, tag="hg", name="hg")
                nc.sync.dma_start(hg.rearrange("p (c j) -> p c j", c=5),
                    scr[hh].rearrange("c (p j) -> p c j", p=128))
                zs = T(hh, "zs", J2)
                V.tensor_add(zs, hg[:, J3 : J3 + J2], qc[:, 2 * J6 + J3 : QW])
                sg = T(hh, "sg", J2)
                nc.scalar.activation(sg, zs, AF.Sigmoid)
                x31v = T(hh, "x31v", J3)
                V.tensor_add(x31v, hg[:, 0:J3], qc[:, 2 * J6 : 2 * J6 + J3])
                d3 = q_["d3"]
                gx = T(hh, "gx", J3); V.tensor_mul(gx, d3, x31v)
                gu = T(hh, "gu", jh)
                V.tensor_add(gu, gx[:, 0:jh], gx[:, jh : 2 * jh])
                V.tensor_add(gu, gu, gx[:, 2 * jh : J3])
                s0t, s1t = sg[:, 0:jh], sg[:, jh:J2]
                ssum = T(hh, "ssum", jh); V.tensor_add(ssum, s0t, s1t)
                sprod = T(hh, "sprod", jh); V.tensor_mul(sprod, s0t, s1t)
                t1 = T(hh, "t1", jh)
                V.scalar_tensor_tensor(t1, ssum, 16.0, q_["bd"], OP.mult, OP.mult)
                t2 = T(hh, "t2", jh)
                V.scalar_tensor_tensor(t2, sprod, 16.0, q_["bar"], OP.mult, OP.mult)
                qa = T(hh, "qa", jh)
                V.scalar_tensor_tensor(qa, gu, 4.0, t1, OP.mult, OP.subtract)
                qb = T(hh, "qb", jh)
                V.scalar_tensor_tensor(qb, q_["Ls"], 12.0, t2, OP.mult, OP.add)
                q = T(hh, "q", jh); V.tensor_sub(q, qa, qb)
                V.tensor_scalar_max(q, q, 0.0)
                lam = T(hh, "lam", jh); V.tensor_mul(lam, q, q_["rg"])
                ui = qpool.tile([128, 3 * jh], FP, tag="ui", name="ui")
                uiv = ui.rearrange("p (j c) -> p c j", c=3)
                for a in range(3):
                    w = T(hh, f"w_{a}", jh)
                    V.tensor_mul(w, lam, d3[:, a * jh : (a + 1) * jh])
                    V.scalar_tensor_tensor(uiv[:, a, :], w, 0.25,
                                           x31v[:, a * jh : (a + 1) * jh],
                                           OP.mult, OP.subtract)
                nc.sync.dma_start(
                    u_d[g0 : g0 + ns // NQ, :].rearrange("(p j) c -> p (j c)", p=128),
                    ui[:])

            # ---------------- MLP: layer-outer phases ----------------
            # All of a layer's inputs are ready before its chunks run, so the
            # ACT stream never stalls on the layer chain.
            hT_all = cpool.tile([H, ns], BF, tag="hT_all", name="hT_all")
            x21a = cpool.tile([H, ns], BF, tag="x21a", name="x21a")
            x22a = cpool.tile([H, ns], BF, tag="x22a", name="x22a")

            def chunk_layer(lhsT, rhs_sl, bias_t, out_sl):
                ps = ppool.tile([128, SC], FP, tag="ps", name="ps")
                for m in range(SC // 512):
                    nc.tensor.matmul(
                        ps[:, 512 * m : 512 * (m + 1)],
                        lhsT,
                        rhs_sl[:, 512 * m : 512 * (m + 1)],
                        start=True, stop=True,
                    )
                nc.scalar.activation(out_sl, ps[:], AF.Tanh, bias=bias_t[:, 0:1])

            def csl(t, i):
                return t[:, SC * i : SC * (i + 1)]

            for i in range(nit):
                xt_c = apool.tile([NF, SC], BF, tag="xt_c", name="xt_c")
                nc.sync.dma_start(xt_c[:], xt_d[:, SC * i : SC * (i + 1)])
                chunk_layer(w1T[:], xt_c[:], bias["b1"], csl(hT_all, i))
            qp_pre(0)
            qp_pre(1)
            for i in range(nit):
                chunk_layer(wts["w21"][:], csl(hT_all, i), bias["b21"], csl(x21a, i))
                chunk_layer(wts["w22"][:], csl(hT_all, i), bias["b22"], csl(x22a, i))

            x21b = cpool.tile([H, ns], BF, tag="x21b", name="x21b")
            for i in range(nit):
                chunk_layer(wts["wm1"][:], csl(x21a, i), bias["bm1"], csl(x21b, i))

            def heads(i):
                s0 = SC * i
                x21 = csl(x21b, i)
                x22 = x22t[i]
                psH = ppool.tile([128, SC], FP, tag="ps", name="ps")
                ps31, ps32 = psH[:, 0:512], psH[:, 512:1024]
                for m in range(4):
                    nc.tensor.matmul(ps31[32 * m : 32 * m + 32, :], wh31[:],
                                     x21[:, 512 * m : 512 * (m + 1)],
                                     start=True, stop=True, tile_position=(0, 32 * m))
                for m in range(4):
                    nc.tensor.matmul(ps32[32 * m : 32 * m + 32, :], wh32[:],
                                     x22[:][:, 512 * m : 512 * (m + 1)],
                                     start=True, stop=True, tile_position=(0, 32 * m))
                hd = hpool.tile([128, 1024], FP, tag="hd", name="hd")
                nc.vector.tensor_copy(hd[:, 0:512], ps31)
                nc.vector.tensor_copy(hd[:, 512:1024], ps32)
                half, off = divmod(s0, ns // NQ)
                for m in range(4):
                    sl = slice(off + 512 * m, off + 512 * (m + 1))
                    eng = nc.sync.dma_start if m % 2 == 0 else nc.gpsimd.dma_start
                    eng(scr[half][0:3, sl], hd[32 * m : 32 * m + 3, 0:512])
                    eng(scr[half][3:5, sl], hd[32 * m : 32 * m + 2, 512:1024])

            x22t = {}
            for i in range(nit):
                x22 = apool.tile([H, SC], BF, tag="x22b", name="x22b")
                chunk_layer(wts["wm2"][:], csl(x22a, i), bias["bm2"], x22[:])
                x22t[i] = x22
                if i > 0:
                    heads(i - 1)
                if i == nit - 1:
                    qp_post(0)
            heads(nit - 1)
            qp_post(1)

    nc.compile()
    return nc


def _get_nc(ns=NS):
    if ns not in _cache:
        _cache[ns] = build(ns)
    return _cache[ns]


def prep_maps(inputs, ns=NS, n_cores=N_CORES):
    """Host-side shard + layout prep. Returns per-core in_maps."""
    f32 = np.float32
    jh = ns // 2 // 128
    g = {k: np.asarray(v) for k, v in inputs.items()}
    x = np.ascontiguousarray(g["x"], f32)
    mean = np.asarray(g["mean"], f32)
    std = np.asarray(g["std"], f32)
    obs = np.array([10.0, 0.0, 10.0, 0.0, 9.0, 0.0], f32)
    moff = mean - obs
    perm = [0, 2, 4, 1, 3, 5]  # pos-block | vel-block order
    qc = np.concatenate([
        np.repeat(std[perm], jh),
        np.repeat(moff[perm], jh),
        np.repeat(np.asarray(g["fc31_b"], f32), jh),
        np.repeat(np.asarray(g["fc32_b"], f32), jh),
    ])
    qc = np.ascontiguousarray(np.broadcast_to(qc, (H, qc.size)))

    def padT(w, cols):
        out = np.zeros((H, 32), f32)
        out[:, :cols] = np.asarray(w, f32).T
        return np.ascontiguousarray(out.astype(BF_NP))

    shared = {
        "w1T": np.ascontiguousarray(np.asarray(g["fc1_w"], f32).T.astype(BF_NP)),
        "w21T": np.ascontiguousarray(np.asarray(g["fc21_w"], f32).T.astype(BF_NP)),
        "w22T": np.ascontiguousarray(np.asarray(g["fc22_w"], f32).T.astype(BF_NP)),
        "wm1T": np.ascontiguousarray(np.asarray(g["fcm1_w"], f32).T.astype(BF_NP)),
        "wm2T": np.ascontiguousarray(np.asarray(g["fcm2_w"], f32).T.astype(BF_NP)),
        "wh31": padT(g["fc31_w"], 3),
        "wh32": padT(g["fc32_w"], 2),
        "b1": np.ascontiguousarray(np.asarray(g["fc1_b"], f32)),
        "b21": np.ascontiguousarray(np.asarray(g["fc21_b"], f32)),
        "b22": np.ascontiguousarray(np.asarray(g["fc22_b"], f32)),
        "bm1": np.ascontiguousarray(np.asarray(g["fcm1_b"], f32)),
        "bm2": np.ascontiguousarray(np.asarray(g["fcm2_b"], f32)),
        "qc": qc,
    }
    in_maps = []
    for c in range(n_cores):
        sh = x[c * ns : (c + 1) * ns]
        m = dict(shared)
        m["x"] = np.ascontiguousarray(sh)
        m["xt"] = np.ascontiguousarray(sh.T.astype(BF_NP))
        in_maps.append(m)
    return in_maps


def kernel(**inputs):
    nc = _get_nc()
    in_maps = prep_maps(inputs)
    res = bass_utils.run_bass_kernel_spmd(nc, in_maps, core_ids=list(range(N_CORES)))
    return np.concatenate([res.results[c]["u"] for c in range(N_CORES)], axis=0)



# revision 3
# speedup vs baseline: 1.2370x; 1.2370x over previous
"""BarrierNet Trainium2 kernel: MLP (6->128->128x2 branches->heads) + closed-form QP.

Data-parallel over 8 cores (16384 samples each). Host pre-shards and
pre-transposes: xt [6,NS] bf16, and packs all weights/biases/QP constants into
two blob tensors (1 bf16 + 1 fp32) loaded with single DMAs. Per core:
  - MLP in transposed layout (hidden on partitions, batch free), 8 chunks of
    2048; each layer = 4 bf16 N=512 matmuls into a 4-bank PSUM group drained
    by ONE fused bias+tanh ACT op (fp32 PSUM -> bf16 SBUF).
  - Heads: per 128-sample block, the activations chunk is the STATIONARY
    matmul operand and the tiny head weight [128,3]/[128,2] is the moving one,
    so the head output lands directly in sample-major layout [128, (j c)] in
    PSUM -- no DRAM transpose round-trip, and only ~5 PE cycles per block.
  - QP runs batched fp32 on DVE in a [128, j] sample grid, reading head
    results straight from PSUM; half 0 is processed while the fcm layers of
    half 1 still drain, so only half 1's QP is exposed as a tail.
"""
import sys

sys.path.insert(0, "/opt/trn_rl_repo")

import numpy as np
import ml_dtypes

import concourse.bacc as bacc
import concourse.bass as bass
import concourse.mybir as mybir
import concourse.tile as tile
from concourse import bass_utils

FP = mybir.dt.float32
BF = mybir.dt.bfloat16
AF = mybir.ActivationFunctionType
OP = mybir.AluOpType
BF_NP = ml_dtypes.bfloat16

N_CORES = 8
B = 131072
NS = B // N_CORES          # samples per core
SC = 2048                  # super-chunk (one PSUM group span)
H = 128
NF = 6
NQ = 2                     # QP halves

# blob_bf columns (bf16)
C_W21, C_W22, C_WM1, C_WM2, C_W1, C_WH1, C_WH2 = 0, 128, 256, 384, 512, 640, 643
BBF_W = 645
# blob_fp columns (fp32): qc | b1 | b21 | b22 | bm1 | bm2

_cache = {}


def build(ns=NS):
    nit = ns // SC
    jh = ns // NQ // 128   # samples per partition in one QP grid
    J3, J2, J6 = 3 * jh, 2 * jh, 6 * jh
    QW = 2 * J6 + J3 + J2
    BFP_W = QW + 5
    half = ns // NQ

    nc = bacc.Bacc("TRN2", target_bir_lowering=False, debug=False)

    x_d = nc.dram_tensor("x", [ns, NF], FP, kind="ExternalInput")
    xt_d = nc.dram_tensor("xt", [NF, ns], BF, kind="ExternalInput")
    bbf_d = nc.dram_tensor("bbf", [H, BBF_W], BF, kind="ExternalInput")
    bfp_d = nc.dram_tensor("bfp", [H, BFP_W], FP, kind="ExternalInput")
    u_d = nc.dram_tensor("u", [ns, 3], FP, kind="ExternalOutput")

    with tile.TileContext(nc) as tc:
        with (
            tc.tile_pool(name="const", bufs=1) as cpool,
            tc.tile_pool(name="act", bufs=2) as apool,
            tc.tile_pool(name="psum", bufs=2, space="PSUM") as ppool,
            tc.tile_pool(name="qp", bufs=2) as qpool,
        ):
            bbf = cpool.tile([H, BBF_W], BF, tag="bbf", name="bbf")
            nc.sync.dma_start(bbf[:], bbf_d[:])
            bfp = cpool.tile([H, BFP_W], FP, tag="bfp", name="bfp")
            nc.sync.dma_start(bfp[:], bfp_d[:])
            qc = bfp[:, 0:QW]

            def bias(k):
                return bfp[:, QW + k : QW + k + 1]

            hT_all = cpool.tile([H, ns], BF, tag="hT_all", name="hT_all")
            x21a = cpool.tile([H, ns], BF, tag="x21a", name="x21a")
            x22a = cpool.tile([H, ns], BF, tag="x22a", name="x22a")
            x21b = cpool.tile([H, ns], BF, tag="x21b", name="x21b")
            x22b = cpool.tile([H, ns], BF, tag="x22b", name="x22b")

            # QP scratch: per-half persistent values + one shared transient
            # region (all DVE work is in-order, so reuse is race-free).
            pers = [cpool.tile([H, 448], FP, tag=f"pers{h}", name=f"pers{h}")
                    for h in range(NQ)]
            scr = cpool.tile([H, 1664], FP, tag="qscr", name="qscr")

            V = nc.vector

            def S(lo, w):
                return scr[:, lo : lo + w]

            # ---------------- QP (sample-grid layout, fp32, batched) --------
            def qp_pre(hh):
                """x-side preamble: needs only x_d -- runs on idle DVE during
                the fc21/fc22 phases."""
                g0 = hh * half
                xg = qpool.tile([128, J6], FP, tag="xg", name="xg")
                nc.sync.dma_start(xg[:],
                    x_d[g0 : g0 + half, :].rearrange("(p j) f -> p (j f)", p=128))
                xgv = xg.rearrange("p (j g e) -> p e g j", g=3, e=2)
                x0 = S(0, J6)
                x0v = x0.rearrange("p (e g j) -> p e g j", e=2, g=3)
                qsv = qc[:, 0:J6].rearrange("p (e g j) -> p e g j", e=2, g=3)
                qmv = qc[:, J6 : 2 * J6].rearrange("p (e g j) -> p e g j", e=2, g=3)
                V.tensor_mul(x0v, xgv, qsv)
                V.tensor_add(x0v, x0v, qmv)
                dd, vv = x0[:, 0:J3], x0[:, J3:J6]
                d3 = pers[hh][:, 0:J3]
                d2 = S(J6, J3); V.tensor_mul(d2, dd, dd)
                V.tensor_mul(d3, d2, dd)
                d4 = S(J6 + J3, J3); V.tensor_mul(d4, d2, d2)
                v2 = S(J6 + 2 * J3, J3); V.tensor_mul(v2, vv, vv)
                dv = S(J6 + 3 * J3, J3); V.tensor_mul(dv, d3, vv)
                dw = S(J6 + 4 * J3, J3); V.tensor_mul(dw, d2, v2)
                d6 = S(J6 + 5 * J3, J3); V.tensor_mul(d6, d3, d3)
                g6 = S(J6 + 6 * J3, jh)

                def a3(t, k):
                    return t[:, k * jh : (k + 1) * jh]

                def sum3(t, r, bias_const=None):
                    V.tensor_add(r, a3(t, 0), a3(t, 1))
                    if bias_const is None:
                        V.tensor_add(r, r, a3(t, 2))
                    else:
                        V.scalar_tensor_tensor(r, r, bias_const, a3(t, 2),
                                               OP.add, OP.add)

                sum3(d4, pers[hh][:, J3 : J3 + jh], -2401.0)   # barrier
                sum3(dv, pers[hh][:, J3 + jh : J3 + 2 * jh])   # barrier_dot / 4
                sum3(dw, pers[hh][:, J3 + 2 * jh : J3 + 3 * jh])  # Lf2b / 12
                sum3(d6, g6)                                   # GG / 16
                V.reciprocal(pers[hh][:, J3 + 3 * jh : J3 + 4 * jh], g6)

            # Stationary-operand views matching the QP grid: partition p of
            # the QP grid holds samples p*jh + j, so head block j must take
            # columns {i*jh + j : i} of the activation tiles (stride jh).
            x21bv = x21b.rearrange("p (q i j) -> p q j i", q=NQ, i=128)
            x22bv = x22b.rearrange("p (q i j) -> p q j i", q=NQ, i=128)

            def heads(hh, psH):
                """Head matmuls, activations stationary: out [128 grid rows,
                (j c)] directly in PSUM, partition-major like the QP grid."""
                for j in range(jh):
                    nc.tensor.matmul(psH[:, 5 * j : 5 * j + 3],
                                     x21bv[:, hh, j, :],
                                     bbf[:, C_WH1 : C_WH1 + 3],
                                     start=True, stop=True)
                    nc.tensor.matmul(psH[:, 5 * j + 3 : 5 * j + 5],
                                     x22bv[:, hh, j, :],
                                     bbf[:, C_WH2 : C_WH2 + 2],
                                     start=True, stop=True)

            def qp_post(hh, psH):
                g0 = hh * half
                p_ = pers[hh]
                d3 = p_[:, 0:J3]
                bar = p_[:, J3 : J3 + jh]
                bd = p_[:, J3 + jh : J3 + 2 * jh]
                Ls = p_[:, J3 + 2 * jh : J3 + 3 * jh]
                rg = p_[:, J3 + 3 * jh : J3 + 4 * jh]
                hgv = psH[:, 0 : 5 * jh].rearrange("p (j c) -> p c j", c=5)
                zs = S(0, J2)
                V.tensor_add(zs.rearrange("p (c j) -> p c j", c=2),
                             hgv[:, 3:5, :],
                             qc[:, 2 * J6 + J3 : QW].rearrange(
                                 "p (c j) -> p c j", c=2))
                sg = S(J2, J2)
                nc.scalar.activation(sg, zs, AF.Sigmoid)
                x31v = S(2 * J2, J3)
                V.tensor_add(x31v.rearrange("p (c j) -> p c j", c=3),
                             hgv[:, 0:3, :],
                             qc[:, 2 * J6 : 2 * J6 + J3].rearrange(
                                 "p (c j) -> p c j", c=3))
                base = 2 * J2 + J3
                gx = S(base, J3); V.tensor_mul(gx, d3, x31v)
                gu = S(base + J3, jh)
                V.tensor_add(gu, gx[:, 0:jh], gx[:, jh : 2 * jh])
                V.tensor_add(gu, gu, gx[:, 2 * jh : J3])
                s0t, s1t = sg[:, 0:jh], sg[:, jh:J2]
                o = base + J3 + jh
                ssum = S(o, jh); V.tensor_add(ssum, s0t, s1t)
                sprod = S(o + jh, jh); V.tensor_mul(sprod, s0t, s1t)
                t1 = S(o + 2 * jh, jh)
                V.scalar_tensor_tensor(t1, ssum, 16.0, bd, OP.mult, OP.mult)
                t2 = S(o + 3 * jh, jh)
                V.scalar_tensor_tensor(t2, sprod, 16.0, bar, OP.mult, OP.mult)
                qa = S(o + 4 * jh, jh)
                V.scalar_tensor_tensor(qa, gu, 4.0, t1, OP.mult, OP.subtract)
                qb = S(o + 5 * jh, jh)
                V.scalar_tensor_tensor(qb, Ls, 12.0, t2, OP.mult, OP.add)
                q = S(o + 6 * jh, jh); V.tensor_sub(q, qa, qb)
                V.tensor_scalar_max(q, q, 0.0)
                lam = S(o + 7 * jh, jh); V.tensor_mul(lam, q, rg)
                ui = qpool.tile([128, J3], FP, tag="ui", name="ui")
                uiv = ui.rearrange("p (j c) -> p c j", c=3)
                w = S(o + 8 * jh, jh)
                for a in range(3):
                    V.tensor_mul(w, lam, d3[:, a * jh : (a + 1) * jh])
                    V.scalar_tensor_tensor(uiv[:, a, :], w, 0.25,
                                           x31v[:, a * jh : (a + 1) * jh],
                                           OP.mult, OP.subtract)
                nc.sync.dma_start(
                    u_d[g0 : g0 + half, :].rearrange("(p j) c -> p (j c)", p=128),
                    ui[:])

            # ---------------- MLP: layer-outer phases ----------------
            def chunk_layer(lhsT, rhs_sl, bias_t, out_sl):
                ps = ppool.tile([128, SC], FP, tag="ps", name="ps")
                for m in range(SC // 512):
                    nc.tensor.matmul(
                        ps[:, 512 * m : 512 * (m + 1)],
                        lhsT,
                        rhs_sl[:, 512 * m : 512 * (m + 1)],
                        start=True, stop=True,
                    )
                nc.scalar.activation(out_sl, ps[:], AF.Tanh, bias=bias_t)

            def csl(t, i):
                return t[:, SC * i : SC * (i + 1)]

            for i in range(nit):
                xt_c = apool.tile([NF, SC], BF, tag="xt_c", name="xt_c")
                eng = nc.gpsimd.dma_start if i % 2 == 0 else nc.sync.dma_start
                eng(xt_c[:], xt_d[:, SC * i : SC * (i + 1)])
                chunk_layer(bbf[0:NF, C_W1 : C_W1 + H], xt_c[:], bias(0),
                            csl(hT_all, i))
            qp_pre(0)
            qp_pre(1)
            for i in range(nit):
                chunk_layer(bbf[:, C_W21 : C_W21 + H], csl(hT_all, i), bias(1),
                            csl(x21a, i))
                chunk_layer(bbf[:, C_W22 : C_W22 + H], csl(hT_all, i), bias(2),
                            csl(x22a, i))

            psH = {}
            for i in range(nit):
                chunk_layer(bbf[:, C_WM1 : C_WM1 + H], csl(x21a, i), bias(3),
                            csl(x21b, i))
                chunk_layer(bbf[:, C_WM2 : C_WM2 + H], csl(x22a, i), bias(4),
                            csl(x22b, i))
                if i == nit // 2:
                    # half 0 fully drained one chunk ago; heads+QP for it run
                    # in PE/DVE idle windows under the remaining ACT drains.
                    psH[0] = ppool.tile([128, SC], FP, tag="ps", name="psH0")
                    heads(0, psH[0])
                    qp_post(0, psH[0])
            psH[1] = ppool.tile([128, SC], FP, tag="ps", name="psH1")
            heads(1, psH[1])
            qp_post(1, psH[1])

    nc.compile()
    return nc


def _get_nc(ns=NS):
    if ns not in _cache:
        _cache[ns] = build(ns)
    return _cache[ns]


def prep_maps(inputs, ns=NS, n_cores=N_CORES):
    """Host-side shard + layout prep. Returns per-core in_maps."""
    f32 = np.float32
    jh = ns // NQ // 128
    g = {k: np.asarray(v) for k, v in inputs.items()}
    x = np.ascontiguousarray(g["x"], f32)
    mean = np.asarray(g["mean"], f32)
    std = np.asarray(g["std"], f32)
    obs = np.array([10.0, 0.0, 10.0, 0.0, 9.0, 0.0], f32)
    moff = mean - obs
    perm = [0, 2, 4, 1, 3, 5]  # pos-block | vel-block order
    qc = np.concatenate([
        np.repeat(std[perm], jh),
        np.repeat(moff[perm], jh),
        np.repeat(np.asarray(g["fc31_b"], f32), jh),
        np.repeat(np.asarray(g["fc32_b"], f32), jh),
    ])
    bfp = np.concatenate([
        np.broadcast_to(qc, (H, qc.size)),
        np.asarray(g["fc1_b"], f32)[:, None],
        np.asarray(g["fc21_b"], f32)[:, None],
        np.asarray(g["fc22_b"], f32)[:, None],
        np.asarray(g["fcm1_b"], f32)[:, None],
        np.asarray(g["fcm2_b"], f32)[:, None],
    ], axis=1)
    w1pad = np.zeros((H, H), f32)
    w1pad[:NF, :] = np.asarray(g["fc1_w"], f32).T
    bbf = np.concatenate([
        np.asarray(g["fc21_w"], f32).T,
        np.asarray(g["fc22_w"], f32).T,
        np.asarray(g["fcm1_w"], f32).T,
        np.asarray(g["fcm2_w"], f32).T,
        w1pad,
        np.asarray(g["fc31_w"], f32).T,
        np.asarray(g["fc32_w"], f32).T,
    ], axis=1).astype(BF_NP)

    shared = {
        "bbf": np.ascontiguousarray(bbf),
        "bfp": np.ascontiguousarray(bfp, f32),
    }
    in_maps = []
    for c in range(n_cores):
        sh = x[c * ns : (c + 1) * ns]
        m = dict(shared)
        m["x"] = np.ascontiguousarray(sh)
        m["xt"] = np.ascontiguousarray(sh.T.astype(BF_NP))
        in_maps.append(m)
    return in_maps


def kernel(**inputs):
    nc = _get_nc()
    in_maps = prep_maps(inputs)
    res = bass_utils.run_bass_kernel_spmd(nc, in_maps, core_ids=list(range(N_CORES)))
    return np.concatenate([res.results[c]["u"] for c in range(N_CORES)], axis=0)
